# revision 30
# baseline (speedup 1.0000x reference)
"""DCN (deep & cross network) inference kernel for 8 trn2 NeuronCores.

Strategy
--------
Data-parallel over the batch: each of the 8 cores processes 2048 of the
16384 rows.  The cross network is collapsed algebraically:

    xl_{i+1} = x0 * (xl_i . w_i) + b_i + xl_i   (x0 = x)
    =>  xl_3 = x * (1 + S) + (b0+b1+b2)

with S a per-row scalar computable from u_i = x . w_i plus constants
c_ij = b_i . w_j.  Only xl_3 . w_out[:1024] feeds the output, so the
whole cross network reduces to 4 per-row dot products u0..u3
(u3 = x . w_out[:1024]) and ~15 scalar ops per row; those dots are a
[16384,1024]x[1024,4] sgemm the host does in fp32 (precision matters
there - the u's multiply each other - and it is 6% of total flops).

The device runs the deep tower in feature-major layout (features on
partitions, rows on the free axis), with BatchNorm folded into the
following matmul's weights/bias:

    Z.T [64, N]  = w1.T @ x.T                     (the 2.1 GFLOP matmul)
    r   [64, N]  = relu(Z.T + b1)
    t2  [48, N]  = tanh(W2'.T @ r + b2')
    t3  [24, N]  = tanh(W3'.T @ t2 + b3')   -> returned per core

Matmuls run in float32r (fp32 rounded to 11 mantissa bits; 1 PE
cycle/column vs fp32's 4) with host-side round-to-nearest-even.  The
relu/tanh chain compresses the ~1e-4 rounding noise, so the final
output error stays at the few-1e-4 level.  x is transposed on the host
so the PE streams it without any on-chip transpose; a burst of dummy
warm-up matmuls during the first x-block's DMA gets the PE HAM clock
gate to 8/8 before real work arrives.  The host finishes with
hd = (a3*w_out_h) . t3, the cross-scalar recurrence and the sigmoid.
"""

import numpy as np

B, D = 16384, 1024
N_CORES = 8
ROWS = B // N_CORES          # rows per core
BS = 512                     # max matmul free-dim block (PSUM bank limit)
# uneven blocks: small final blocks shorten the end-of-kernel serial tail
BLOCKS = [512, 512, 512, 256, 256]
NBLK = len(BLOCKS)
KT = D // 128                # number of 128-feature contraction tiles
NW = 64                      # tower width
N_WARMUP = 6                 # dummy matmuls to warm the PE clock gate
CH = 4                       # k-tiles per x DMA chunk
NCH = KT // CH
EPS = 1e-3

# (offset, size, chunk_flat_offset, n_chunks) per block; x is packed
# chunk-contiguous on the host so every chunk DMA is one contiguous region.
# Small blocks use a single chunk so the Sync ring stays at <= 8 DMAs
# (more would stall on semaphore-lane recycling).
_BLK = []
_off = 0
_flat = 0
for _bs in BLOCKS:
    _BLK.append((_off, _bs, _flat, NCH if _bs == BS else 1))
    _off += _bs
    _flat += KT * 128 * _bs
XT_ELEMS = _flat             # == D * ROWS

# const layout inside the fused weight tensor [128, CW]
_W2_OFF = KT * NW            # 512
_W3_OFF = _W2_OFF + 48       # 560
_B_OFF = _W3_OFF + 24        # 584: b1, b2', b3' as f32 bit-pairs
CW = _B_OFF + 6              # 590

_STATE: dict = {}


def _round_fp32r(a: np.ndarray) -> np.ndarray:
    """Round-to-nearest-even fp32 -> fp32r (low 12 mantissa bits zero)."""
    u = np.ascontiguousarray(a, np.float32).view(np.uint32).copy()
    u += 0x7FF + ((u >> 12) & 1)
    u &= np.uint32(0xFFFFF000)
    return u.view(np.float32)


def _build_bass():
    import concourse.bacc as bacc
    import concourse.bass as bass
    import concourse.mybir as mybir
    import concourse.tile as tile

    f32 = mybir.dt.float32
    f32r = mybir.dt.float32r
    f16 = mybir.dt.float16
    AFT = mybir.ActivationFunctionType

    nc = bacc.Bacc("TRN2", target_bir_lowering=False, debug=False)

    xt = nc.dram_tensor("xt", [XT_ELEMS], f16, kind="ExternalInput")
    wts = nc.dram_tensor("wts", [128, CW], f16, kind="ExternalInput")
    out2 = nc.dram_tensor("out2", [48, ROWS], f32, kind="ExternalOutput")

    with tile.TileContext(nc) as tc:
        with (
            tc.tile_pool(name="const", bufs=1) as cpool,
            tc.tile_pool(name="xin", bufs=16) as xpool,
            tc.tile_pool(name="act", bufs=4) as apool,
            tc.tile_pool(name="pz", bufs=3, space=bass.MemorySpace.PSUM) as pz,
            tc.tile_pool(name="p2", bufs=2, space=bass.MemorySpace.PSUM) as p2,
            tc.tile_pool(name="pw", bufs=1, space=bass.MemorySpace.PSUM) as pw,
        ):
            w_t = cpool.tile([128, CW], f16)
            nc.scalar.dma_start(w_t[:], wts[:])

            W2 = w_t[0:64, _W2_OFF:_W2_OFF + 48]
            B1 = w_t[0:64, _B_OFF + 0:_B_OFF + 2].bitcast(f32)
            B2 = w_t[0:48, _B_OFF + 2:_B_OFF + 4].bitcast(f32)

            def wk(k):
                return w_t[:, k * NW:(k + 1) * NW]

            # PE warm-up: dummy matmuls on the (already loaded) weights so
            # the HAM clock gate reaches 8/8 while the first x block DMAs.
            wm = pw.tile([NW, BS], f32)
            for _ in range(N_WARMUP):
                nc.tensor.matmul(wm[:], wk(0), w_t[:, 0:BS], start=True, stop=True)

            xt_f = xt.ap()  # flat fp16, chunk-contiguous host packing

            rs: dict = {}

            def tower2(i):
                # mm2 + tanh for block i (relu(i) finished a block ago, so
                # the PE never stalls on the activation chain)
                r, off, bs = rs[i]
                z2 = p2.tile([48, bs], f32, tag="z2")
                nc.tensor.matmul(z2[:], W2, r[:], start=True, stop=True)
                t2 = apool.tile([48, bs], f32, tag="t2")
                nc.scalar.activation(t2[:], z2[:], AFT.Tanh, bias=B2)
                nc.scalar.dma_start(out2[:, off:off + bs], t2[:])

            for b, (off, bs, flat, nch) in enumerate(_BLK):
                # stream the block in k-tile chunks so the PE starts as
                # soon as the first chunk lands and DMA never stalls
                chunks = []
                ch = KT // nch
                csz = 128 * ch * bs
                for j in range(nch):
                    xc = xpool.tile([128, ch, bs], f16, tag="xc")
                    src = xt_f[flat + j * csz: flat + (j + 1) * csz]
                    nc.sync.dma_start(
                        xc[:], src.rearrange("(p k n) -> p k n", p=128, k=ch))
                    chunks.append(xc)

                zt = pz.tile([NW, bs], f32, tag="zt")
                for k in range(KT):
                    nc.tensor.matmul(
                        zt[:], wk(k), chunks[k // ch][:, k % ch, :],
                        start=(k == 0), stop=(k == KT - 1),
                    )

                r = apool.tile([64, bs], f16, tag="r")
                nc.vector.tensor_scalar(
                    r[:], zt[:], B1, 0.0,
                    mybir.AluOpType.add, mybir.AluOpType.max,
                )
                rs[b] = (r, off, bs)

                if b >= 1:
                    tower2(b - 1)

            tower2(NBLK - 1)

    nc.compile()
    return nc


def _get_nc():
    if "nc" not in _STATE:
        _STATE["nc"] = _build_bass()
    return _STATE["nc"]


def _prep(inputs):
    """Host-side folding of the tiny weights + the fp32 u-sgemm."""
    f32 = np.float32
    x = np.asarray(inputs["x"], f32)
    cw = np.asarray(inputs["cross_w"], f32)
    cb = np.asarray(inputs["cross_b"], f32)
    w1 = np.asarray(inputs["w1"], f32)
    b1 = np.asarray(inputs["b1"], f32)
    w2 = np.asarray(inputs["w2"], f32)
    b2 = np.asarray(inputs["b2"], f32)
    w3 = np.asarray(inputs["w3"], f32)
    b3 = np.asarray(inputs["b3"], f32)
    w_out = np.asarray(inputs["w_out"], f32)
    b_out = np.asarray(inputs["b_out"], f32)

    def bn_fold(g, be, m, v):
        a = (np.asarray(g, np.float64) / np.sqrt(np.asarray(v, np.float64) + EPS))
        c = np.asarray(be, np.float64) - a * np.asarray(m, np.float64)
        return a, c

    a1, c1 = bn_fold(inputs["gamma1"], inputs["beta1"], inputs["mean1"], inputs["var1"])
    a2, c2 = bn_fold(inputs["gamma2"], inputs["beta2"], inputs["mean2"], inputs["var2"])
    a3, c3 = bn_fold(inputs["gamma3"], inputs["beta3"], inputs["mean3"], inputs["var3"])

    w_out_x = w_out[:D, 0]
    w_out_h = w_out[D:, 0]

    W2p = (a1[:, None] * w2).astype(f32)                  # [64, 48]
    b2p = (c1 @ w2 + b2).astype(f32)                      # [48]
    W3p = (a2[:, None] * w3).astype(f32)                  # [48, 24]
    b3p = (c2 @ w3 + b3).astype(f32)                      # [24]
    wh = (a3 * w_out_h).astype(f32)                       # [24]
    ch = float(c3 @ w_out_h)

    c01 = float(cb[0] @ cw[1])
    c02 = float(cb[0] @ cw[2])
    c12 = float(cb[1] @ cw[2])
    c3s = float(cb.sum(axis=0) @ w_out_x)

    # the 4 cross dot products, exact fp32 on host (6% of total flops)
    Wc = np.stack([cw[0], cw[1], cw[2], w_out_x], axis=1).astype(f32)   # [D, 4]
    U = x @ Wc                                                          # [B, 4]

    # fused device-side const tensor (fp16)
    wts = np.zeros((128, CW), np.float16)
    wts[:, :KT * NW] = w1.astype(np.float16).reshape(
        KT, 128, NW).transpose(1, 0, 2).reshape(128, -1)
    wts[0:64, _W2_OFF:_W2_OFF + 48] = W2p.astype(np.float16)
    wts[0:48, _W3_OFF:_W3_OFF + 24] = W3p.astype(np.float16)
    wts32 = wts.view(np.float32)
    wts32[0:64, (_B_OFF + 0) // 2] = b1
    wts32[0:48, (_B_OFF + 2) // 2] = b2p
    wts32[0:24, (_B_OFF + 4) // 2] = b3p

    consts = dict(c01=c01, c02=c02, c12=c12, c3s=c3s, ch=ch,
                  b_out=float(b_out[0]), wh=wh, U=U, W3p=W3p, b3p=b3p)
    return x, wts, consts


def _combine(t2_all, consts):
    """t2_all: [48, B] device tower output -> final sigmoid output [B, 1].

    The host finishes the tiny third layer (16k x 48 x 24 sgemm + tanh),
    the cross-scalar recurrence and the sigmoid."""
    t3 = np.tanh(consts["W3p"].T @ t2_all + consts["b3p"][:, None])      # [24, B]
    hd = consts["wh"].astype(np.float64) @ t3.astype(np.float64)         # [B]
    U = consts["U"].astype(np.float64)
    u0, u1, u2, u3 = U[:, 0], U[:, 1], U[:, 2], U[:, 3]
    oneS = ((1.0 + u0) * (1.0 + u1) + consts["c01"]) * (1.0 + u2) \
        + consts["c02"] + consts["c12"]
    lin = oneS * u3 + consts["c3s"] + hd + consts["ch"] + consts["b_out"]
    y = 1.0 / (1.0 + np.exp(-lin))
    return y.reshape(-1, 1).astype(np.float32)


def _run(inputs, trace=False, **spmd_kwargs):
    from concourse.bass_utils import run_bass_kernel_spmd

    x, wts, consts = _prep(inputs)
    nc = _get_nc()

    x16 = x.astype(np.float16).reshape(N_CORES, ROWS, KT, 128)
    in_maps = []
    for c in range(N_CORES):
        # chunk-contiguous packing: for each block and k-chunk, a flat
        # [128, CH, bs] slab so each chunk DMA is one contiguous region
        parts = []
        for off, bs, _, nch in _BLK:
            blk = x16[c, off:off + bs]           # [bs, KT, 128]
            ch = KT // nch
            for j in range(nch):
                parts.append(
                    blk[:, j * ch:(j + 1) * ch, :].transpose(2, 1, 0).ravel())
        in_maps.append({"xt": np.concatenate(parts), "wts": wts})

    res = run_bass_kernel_spmd(
        nc, in_maps, core_ids=list(range(N_CORES)), trace=trace, **spmd_kwargs
    )
    t2_all = np.concatenate([r["out2"] for r in res.results], axis=1)  # [48, B]
    return _combine(t2_all, consts), res


def kernel(**inputs) -> np.ndarray:
    y, _ = _run(inputs, trace=False)
    return y


# revision 31
# speedup vs baseline: 1.0441x; 1.0441x over previous
"""DCN (deep & cross network) inference kernel for 8 trn2 NeuronCores.

Strategy
--------
Data-parallel over the batch: each of the 8 cores processes 2048 of the
16384 rows.  The cross network is collapsed algebraically:

    xl_{i+1} = x0 * (xl_i . w_i) + b_i + xl_i   (x0 = x)
    =>  xl_3 = x * (1 + S) + (b0+b1+b2)

with S a per-row scalar computable from u_i = x . w_i plus constants
c_ij = b_i . w_j.  Only xl_3 . w_out[:1024] feeds the output, so the
whole cross network reduces to 4 per-row dot products u0..u3
(u3 = x . w_out[:1024]) and ~15 scalar ops per row; those dots are a
[16384,1024]x[1024,4] sgemm the host does in fp32 (precision matters
there - the u's multiply each other - and it is 6% of total flops).

The device runs the deep tower in feature-major layout (features on
partitions, rows on the free axis), with BatchNorm folded into the
following matmul's weights/bias:

    Z.T [64, N]  = w1.T @ x.T                     (the 2.1 GFLOP matmul)
    r   [64, N]  = relu(Z.T + b1)
    t2  [48, N]  = tanh(W2'.T @ r + b2')
    t3  [24, N]  = tanh(W3'.T @ t2 + b3')   -> returned per core

Matmuls run in float32r (fp32 rounded to 11 mantissa bits; 1 PE
cycle/column vs fp32's 4) with host-side round-to-nearest-even.  The
relu/tanh chain compresses the ~1e-4 rounding noise, so the final
output error stays at the few-1e-4 level.  x is transposed on the host
so the PE streams it without any on-chip transpose; a burst of dummy
warm-up matmuls during the first x-block's DMA gets the PE HAM clock
gate to 8/8 before real work arrives.  The host finishes with
hd = (a3*w_out_h) . t3, the cross-scalar recurrence and the sigmoid.
"""

import numpy as np

B, D = 16384, 1024
N_CORES = 8
ROWS = B // N_CORES          # rows per core
BS = 512                     # max matmul free-dim block (PSUM bank limit)
# uneven blocks: small final blocks shorten the end-of-kernel serial tail
BLOCKS = [512, 512, 512, 256, 256]
NBLK = len(BLOCKS)
KT = D // 128                # number of 128-feature contraction tiles
NW = 64                      # tower width
N_WARMUP = 6                 # dummy matmuls to warm the PE clock gate
CH = 4                       # k-tiles per x DMA chunk
NCH = KT // CH
EPS = 1e-3

# (offset, size, chunk_flat_offset, n_chunks) per block; x is packed
# chunk-contiguous on the host so every chunk DMA is one contiguous region.
# Small blocks use a single chunk so the Sync ring stays at <= 8 DMAs
# (more would stall on semaphore-lane recycling).
_BLK = []
_off = 0
_flat = 0
for _bs in BLOCKS:
    _BLK.append((_off, _bs, _flat, NCH if _bs == BS else 1))
    _off += _bs
    _flat += KT * 128 * _bs
XT_ELEMS = _flat             # == D * ROWS

# const layout inside the fused weight tensor [128, CW]
_W2_OFF = KT * NW            # 512
_W3_OFF = _W2_OFF + 48       # 560
_B_OFF = _W3_OFF + 24        # 584: b1, b2', b3' as f32 bit-pairs
CW = _B_OFF + 6              # 590

_STATE: dict = {}


def _round_fp32r(a: np.ndarray) -> np.ndarray:
    """Round-to-nearest-even fp32 -> fp32r (low 12 mantissa bits zero)."""
    u = np.ascontiguousarray(a, np.float32).view(np.uint32).copy()
    u += 0x7FF + ((u >> 12) & 1)
    u &= np.uint32(0xFFFFF000)
    return u.view(np.float32)


def _build_bass():
    import concourse.bacc as bacc
    import concourse.bass as bass
    import concourse.mybir as mybir
    import concourse.tile as tile

    f32 = mybir.dt.float32
    f32r = mybir.dt.float32r
    f16 = mybir.dt.float16
    AFT = mybir.ActivationFunctionType

    nc = bacc.Bacc("TRN2", target_bir_lowering=False, debug=False)

    xt = nc.dram_tensor("xt", [XT_ELEMS], f16, kind="ExternalInput")
    wts = nc.dram_tensor("wts", [128, CW], f16, kind="ExternalInput")
    out2 = nc.dram_tensor("out2", [48, ROWS], f32, kind="ExternalOutput")

    with tile.TileContext(nc) as tc:
        with (
            tc.tile_pool(name="const", bufs=1) as cpool,
            tc.tile_pool(name="xin", bufs=16) as xpool,
            tc.tile_pool(name="act", bufs=4) as apool,
            tc.tile_pool(name="pz", bufs=3, space=bass.MemorySpace.PSUM) as pz,
            tc.tile_pool(name="p2", bufs=2, space=bass.MemorySpace.PSUM) as p2,
            tc.tile_pool(name="pw", bufs=1, space=bass.MemorySpace.PSUM) as pw,
        ):
            w_t = cpool.tile([128, CW], f16)
            nc.scalar.dma_start(w_t[:], wts[:])

            W2 = w_t[0:64, _W2_OFF:_W2_OFF + 48]
            B1 = w_t[0:64, _B_OFF + 0:_B_OFF + 2].bitcast(f32)
            B2 = w_t[0:48, _B_OFF + 2:_B_OFF + 4].bitcast(f32)

            def wk(k):
                return w_t[:, k * NW:(k + 1) * NW]

            # PE warm-up: dummy matmuls on a zeroed tile (no DMA dependency,
            # so they start right after the preamble) to get the HAM clock
            # gate to 8/8 before the first real matmul.
            zeros = cpool.tile([128, BS], f16)
            nc.vector.memset(zeros[:], 0.0)
            wm = pw.tile([NW, BS], f32)
            for _ in range(N_WARMUP):
                nc.tensor.matmul(wm[:], zeros[:, 0:NW], zeros[:], start=True,
                                 stop=True)

            xt_f = xt.ap()  # flat fp16, chunk-contiguous host packing

            rs: dict = {}

            def tower2(i):
                # mm2 + tanh for block i (relu(i) finished a block ago, so
                # the PE never stalls on the activation chain)
                r, off, bs = rs[i]
                z2 = p2.tile([48, bs], f32, tag="z2")
                nc.tensor.matmul(z2[:], W2, r[:], start=True, stop=True)
                t2 = apool.tile([48, bs], f32, tag="t2")
                nc.scalar.activation(t2[:], z2[:], AFT.Tanh, bias=B2)
                nc.scalar.dma_start(out2[:, off:off + bs], t2[:])

            for b, (off, bs, flat, nch) in enumerate(_BLK):
                # stream the block in k-tile chunks so the PE starts as
                # soon as the first chunk lands and DMA never stalls
                chunks = []
                ch = KT // nch
                csz = 128 * ch * bs
                for j in range(nch):
                    xc = xpool.tile([128, ch, bs], f16, tag="xc")
                    src = xt_f[flat + j * csz: flat + (j + 1) * csz]
                    nc.sync.dma_start(
                        xc[:], src.rearrange("(p k n) -> p k n", p=128, k=ch))
                    chunks.append(xc)

                zt = pz.tile([NW, bs], f32, tag="zt")
                for k in range(KT):
                    nc.tensor.matmul(
                        zt[:], wk(k), chunks[k // ch][:, k % ch, :],
                        start=(k == 0), stop=(k == KT - 1),
                    )

                r = apool.tile([64, bs], f16, tag="r")
                nc.vector.tensor_scalar(
                    r[:], zt[:], B1, 0.0,
                    mybir.AluOpType.add, mybir.AluOpType.max,
                )
                rs[b] = (r, off, bs)

                if b >= 1:
                    tower2(b - 1)

            tower2(NBLK - 1)

    nc.compile()
    return nc


def _get_nc():
    if "nc" not in _STATE:
        _STATE["nc"] = _build_bass()
    return _STATE["nc"]


def _prep(inputs):
    """Host-side folding of the tiny weights + the fp32 u-sgemm."""
    f32 = np.float32
    x = np.asarray(inputs["x"], f32)
    cw = np.asarray(inputs["cross_w"], f32)
    cb = np.asarray(inputs["cross_b"], f32)
    w1 = np.asarray(inputs["w1"], f32)
    b1 = np.asarray(inputs["b1"], f32)
    w2 = np.asarray(inputs["w2"], f32)
    b2 = np.asarray(inputs["b2"], f32)
    w3 = np.asarray(inputs["w3"], f32)
    b3 = np.asarray(inputs["b3"], f32)
    w_out = np.asarray(inputs["w_out"], f32)
    b_out = np.asarray(inputs["b_out"], f32)

    def bn_fold(g, be, m, v):
        a = (np.asarray(g, np.float64) / np.sqrt(np.asarray(v, np.float64) + EPS))
        c = np.asarray(be, np.float64) - a * np.asarray(m, np.float64)
        return a, c

    a1, c1 = bn_fold(inputs["gamma1"], inputs["beta1"], inputs["mean1"], inputs["var1"])
    a2, c2 = bn_fold(inputs["gamma2"], inputs["beta2"], inputs["mean2"], inputs["var2"])
    a3, c3 = bn_fold(inputs["gamma3"], inputs["beta3"], inputs["mean3"], inputs["var3"])

    w_out_x = w_out[:D, 0]
    w_out_h = w_out[D:, 0]

    W2p = (a1[:, None] * w2).astype(f32)                  # [64, 48]
    b2p = (c1 @ w2 + b2).astype(f32)                      # [48]
    W3p = (a2[:, None] * w3).astype(f32)                  # [48, 24]
    b3p = (c2 @ w3 + b3).astype(f32)                      # [24]
    wh = (a3 * w_out_h).astype(f32)                       # [24]
    ch = float(c3 @ w_out_h)

    c01 = float(cb[0] @ cw[1])
    c02 = float(cb[0] @ cw[2])
    c12 = float(cb[1] @ cw[2])
    c3s = float(cb.sum(axis=0) @ w_out_x)

    # the 4 cross dot products, exact fp32 on host (6% of total flops)
    Wc = np.stack([cw[0], cw[1], cw[2], w_out_x], axis=1).astype(f32)   # [D, 4]
    U = x @ Wc                                                          # [B, 4]

    # fused device-side const tensor (fp16)
    wts = np.zeros((128, CW), np.float16)
    wts[:, :KT * NW] = w1.astype(np.float16).reshape(
        KT, 128, NW).transpose(1, 0, 2).reshape(128, -1)
    wts[0:64, _W2_OFF:_W2_OFF + 48] = W2p.astype(np.float16)
    wts[0:48, _W3_OFF:_W3_OFF + 24] = W3p.astype(np.float16)
    wts32 = wts.view(np.float32)
    wts32[0:64, (_B_OFF + 0) // 2] = b1
    wts32[0:48, (_B_OFF + 2) // 2] = b2p
    wts32[0:24, (_B_OFF + 4) // 2] = b3p

    consts = dict(c01=c01, c02=c02, c12=c12, c3s=c3s, ch=ch,
                  b_out=float(b_out[0]), wh=wh, U=U, W3p=W3p, b3p=b3p)
    return x, wts, consts


def _combine(t2_all, consts):
    """t2_all: [48, B] device tower output -> final sigmoid output [B, 1].

    The host finishes the tiny third layer (16k x 48 x 24 sgemm + tanh),
    the cross-scalar recurrence and the sigmoid."""
    t3 = np.tanh(consts["W3p"].T @ t2_all + consts["b3p"][:, None])      # [24, B]
    hd = consts["wh"].astype(np.float64) @ t3.astype(np.float64)         # [B]
    U = consts["U"].astype(np.float64)
    u0, u1, u2, u3 = U[:, 0], U[:, 1], U[:, 2], U[:, 3]
    oneS = ((1.0 + u0) * (1.0 + u1) + consts["c01"]) * (1.0 + u2) \
        + consts["c02"] + consts["c12"]
    lin = oneS * u3 + consts["c3s"] + hd + consts["ch"] + consts["b_out"]
    y = 1.0 / (1.0 + np.exp(-lin))
    return y.reshape(-1, 1).astype(np.float32)


def _run(inputs, trace=False, **spmd_kwargs):
    from concourse.bass_utils import run_bass_kernel_spmd

    x, wts, consts = _prep(inputs)
    nc = _get_nc()

    x16 = x.astype(np.float16).reshape(N_CORES, ROWS, KT, 128)
    in_maps = []
    for c in range(N_CORES):
        # chunk-contiguous packing: for each block and k-chunk, a flat
        # [128, CH, bs] slab so each chunk DMA is one contiguous region
        parts = []
        for off, bs, _, nch in _BLK:
            blk = x16[c, off:off + bs]           # [bs, KT, 128]
            ch = KT // nch
            for j in range(nch):
                parts.append(
                    blk[:, j * ch:(j + 1) * ch, :].transpose(2, 1, 0).ravel())
        in_maps.append({"xt": np.concatenate(parts), "wts": wts})

    res = run_bass_kernel_spmd(
        nc, in_maps, core_ids=list(range(N_CORES)), trace=trace, **spmd_kwargs
    )
    t2_all = np.concatenate([r["out2"] for r in res.results], axis=1)  # [48, B]
    return _combine(t2_all, consts), res


def kernel(**inputs) -> np.ndarray:
    y, _ = _run(inputs, trace=False)
    return y


# revision 32
# speedup vs baseline: 1.0577x; 1.0131x over previous
"""DCN (deep & cross network) inference kernel for 8 trn2 NeuronCores.

Strategy
--------
Data-parallel over the batch: each of the 8 cores processes 2048 of the
16384 rows.  The cross network is collapsed algebraically:

    xl_{i+1} = x0 * (xl_i . w_i) + b_i + xl_i   (x0 = x)
    =>  xl_3 = x * (1 + S) + (b0+b1+b2)

with S a per-row scalar computable from u_i = x . w_i plus constants
c_ij = b_i . w_j.  Only xl_3 . w_out[:1024] feeds the output, so the
whole cross network reduces to 4 per-row dot products u0..u3
(u3 = x . w_out[:1024]) and ~15 scalar ops per row; those dots are a
[16384,1024]x[1024,4] sgemm the host does in fp32 (precision matters
there - the u's multiply each other - and it is 6% of total flops).

The device runs the deep tower in feature-major layout (features on
partitions, rows on the free axis), with BatchNorm folded into the
following matmul's weights/bias:

    Z.T [64, N]  = w1.T @ x.T                     (the 2.1 GFLOP matmul)
    r   [64, N]  = relu(Z.T + b1)
    t2  [48, N]  = tanh(W2'.T @ r + b2')
    t3  [24, N]  = tanh(W3'.T @ t2 + b3')   -> returned per core

Matmuls run in float32r (fp32 rounded to 11 mantissa bits; 1 PE
cycle/column vs fp32's 4) with host-side round-to-nearest-even.  The
relu/tanh chain compresses the ~1e-4 rounding noise, so the final
output error stays at the few-1e-4 level.  x is transposed on the host
so the PE streams it without any on-chip transpose; a burst of dummy
warm-up matmuls during the first x-block's DMA gets the PE HAM clock
gate to 8/8 before real work arrives.  The host finishes with
hd = (a3*w_out_h) . t3, the cross-scalar recurrence and the sigmoid.
"""

import numpy as np

B, D = 16384, 1024
N_CORES = 8
ROWS = B // N_CORES          # rows per core
BS = 512                     # max matmul free-dim block (PSUM bank limit)
# uneven blocks: small final blocks shorten the end-of-kernel serial tail
BLOCKS = [512, 512, 512, 256, 256]
NBLK = len(BLOCKS)
KT = D // 128                # number of 128-feature contraction tiles
NW = 64                      # tower width
N_WARMUP = 10                 # dummy matmuls to warm the PE clock gate
CH = 4                       # k-tiles per x DMA chunk
NCH = KT // CH
EPS = 1e-3

# (offset, size, chunk_flat_offset, n_chunks) per block; x is packed
# chunk-contiguous on the host so every chunk DMA is one contiguous region.
# Small blocks use a single chunk so the Sync ring stays at <= 8 DMAs
# (more would stall on semaphore-lane recycling).
_BLK = []
_off = 0
_flat = 0
for _bs in BLOCKS:
    _BLK.append((_off, _bs, _flat, NCH if _bs == BS else 1))
    _off += _bs
    _flat += KT * 128 * _bs
XT_ELEMS = _flat             # == D * ROWS

# const layout inside the fused weight tensor [128, CW]
_W2_OFF = KT * NW            # 512
_W3_OFF = _W2_OFF + 48       # 560
_B_OFF = _W3_OFF + 24        # 584: b1, b2', b3' as f32 bit-pairs
CW = _B_OFF + 6              # 590

_STATE: dict = {}


def _round_fp32r(a: np.ndarray) -> np.ndarray:
    """Round-to-nearest-even fp32 -> fp32r (low 12 mantissa bits zero)."""
    u = np.ascontiguousarray(a, np.float32).view(np.uint32).copy()
    u += 0x7FF + ((u >> 12) & 1)
    u &= np.uint32(0xFFFFF000)
    return u.view(np.float32)


def _build_bass():
    import concourse.bacc as bacc
    import concourse.bass as bass
    import concourse.mybir as mybir
    import concourse.tile as tile

    f32 = mybir.dt.float32
    f32r = mybir.dt.float32r
    f16 = mybir.dt.float16
    AFT = mybir.ActivationFunctionType

    nc = bacc.Bacc("TRN2", target_bir_lowering=False, debug=False)

    xt = nc.dram_tensor("xt", [XT_ELEMS], f16, kind="ExternalInput")
    wts = nc.dram_tensor("wts", [128, CW], f16, kind="ExternalInput")
    out2 = nc.dram_tensor("out2", [48, ROWS], f32, kind="ExternalOutput")

    with tile.TileContext(nc) as tc:
        with (
            tc.tile_pool(name="const", bufs=1) as cpool,
            tc.tile_pool(name="xin", bufs=16) as xpool,
            tc.tile_pool(name="act", bufs=4) as apool,
            tc.tile_pool(name="pz", bufs=3, space=bass.MemorySpace.PSUM) as pz,
            tc.tile_pool(name="p2", bufs=2, space=bass.MemorySpace.PSUM) as p2,
            tc.tile_pool(name="pw", bufs=1, space=bass.MemorySpace.PSUM) as pw,
        ):
            w_t = cpool.tile([128, CW], f16)
            nc.scalar.dma_start(w_t[:], wts[:])

            W2 = w_t[0:64, _W2_OFF:_W2_OFF + 48]
            B1 = w_t[0:64, _B_OFF + 0:_B_OFF + 2].bitcast(f32)
            B2 = w_t[0:48, _B_OFF + 2:_B_OFF + 4].bitcast(f32)

            def wk(k):
                return w_t[:, k * NW:(k + 1) * NW]

            # PE warm-up: dummy matmuls on a zeroed tile (no DMA dependency,
            # so they start right after the preamble) to get the HAM clock
            # gate to 8/8 before the first real matmul.
            zeros = cpool.tile([128, BS], f16)
            nc.vector.memset(zeros[:], 0.0)
            wm = pw.tile([NW, BS], f32)
            for _ in range(N_WARMUP):
                nc.tensor.matmul(wm[:], zeros[:, 0:NW], zeros[:], start=True,
                                 stop=True)

            xt_f = xt.ap()  # flat fp16, chunk-contiguous host packing

            rs: dict = {}

            def tower2(i):
                # mm2 + tanh for block i (relu(i) finished a block ago, so
                # the PE never stalls on the activation chain)
                r, off, bs = rs[i]
                z2 = p2.tile([48, bs], f32, tag="z2")
                nc.tensor.matmul(z2[:], W2, r[:], start=True, stop=True)
                t2 = apool.tile([48, bs], f32, tag="t2")
                nc.scalar.activation(t2[:], z2[:], AFT.Tanh, bias=B2)
                nc.scalar.dma_start(out2[:, off:off + bs], t2[:])

            for b, (off, bs, flat, nch) in enumerate(_BLK):
                # stream the block in k-tile chunks so the PE starts as
                # soon as the first chunk lands and DMA never stalls
                chunks = []
                ch = KT // nch
                csz = 128 * ch * bs
                for j in range(nch):
                    xc = xpool.tile([128, ch, bs], f16, tag="xc")
                    src = xt_f[flat + j * csz: flat + (j + 1) * csz]
                    nc.sync.dma_start(
                        xc[:], src.rearrange("(p k n) -> p k n", p=128, k=ch))
                    chunks.append(xc)

                zt = pz.tile([NW, bs], f32, tag="zt")
                for k in range(KT):
                    nc.tensor.matmul(
                        zt[:], wk(k), chunks[k // ch][:, k % ch, :],
                        start=(k == 0), stop=(k == KT - 1),
                    )

                r = apool.tile([64, bs], f16, tag="r")
                nc.vector.tensor_scalar(
                    r[:], zt[:], B1, 0.0,
                    mybir.AluOpType.add, mybir.AluOpType.max,
                )
                rs[b] = (r, off, bs)

                if b >= 1:
                    tower2(b - 1)

            tower2(NBLK - 1)

    nc.compile()
    return nc


def _get_nc():
    if "nc" not in _STATE:
        _STATE["nc"] = _build_bass()
    return _STATE["nc"]


def _prep(inputs):
    """Host-side folding of the tiny weights + the fp32 u-sgemm."""
    f32 = np.float32
    x = np.asarray(inputs["x"], f32)
    cw = np.asarray(inputs["cross_w"], f32)
    cb = np.asarray(inputs["cross_b"], f32)
    w1 = np.asarray(inputs["w1"], f32)
    b1 = np.asarray(inputs["b1"], f32)
    w2 = np.asarray(inputs["w2"], f32)
    b2 = np.asarray(inputs["b2"], f32)
    w3 = np.asarray(inputs["w3"], f32)
    b3 = np.asarray(inputs["b3"], f32)
    w_out = np.asarray(inputs["w_out"], f32)
    b_out = np.asarray(inputs["b_out"], f32)

    def bn_fold(g, be, m, v):
        a = (np.asarray(g, np.float64) / np.sqrt(np.asarray(v, np.float64) + EPS))
        c = np.asarray(be, np.float64) - a * np.asarray(m, np.float64)
        return a, c

    a1, c1 = bn_fold(inputs["gamma1"], inputs["beta1"], inputs["mean1"], inputs["var1"])
    a2, c2 = bn_fold(inputs["gamma2"], inputs["beta2"], inputs["mean2"], inputs["var2"])
    a3, c3 = bn_fold(inputs["gamma3"], inputs["beta3"], inputs["mean3"], inputs["var3"])

    w_out_x = w_out[:D, 0]
    w_out_h = w_out[D:, 0]

    W2p = (a1[:, None] * w2).astype(f32)                  # [64, 48]
    b2p = (c1 @ w2 + b2).astype(f32)                      # [48]
    W3p = (a2[:, None] * w3).astype(f32)                  # [48, 24]
    b3p = (c2 @ w3 + b3).astype(f32)                      # [24]
    wh = (a3 * w_out_h).astype(f32)                       # [24]
    ch = float(c3 @ w_out_h)

    c01 = float(cb[0] @ cw[1])
    c02 = float(cb[0] @ cw[2])
    c12 = float(cb[1] @ cw[2])
    c3s = float(cb.sum(axis=0) @ w_out_x)

    # the 4 cross dot products, exact fp32 on host (6% of total flops)
    Wc = np.stack([cw[0], cw[1], cw[2], w_out_x], axis=1).astype(f32)   # [D, 4]
    U = x @ Wc                                                          # [B, 4]

    # fused device-side const tensor (fp16)
    wts = np.zeros((128, CW), np.float16)
    wts[:, :KT * NW] = w1.astype(np.float16).reshape(
        KT, 128, NW).transpose(1, 0, 2).reshape(128, -1)
    wts[0:64, _W2_OFF:_W2_OFF + 48] = W2p.astype(np.float16)
    wts[0:48, _W3_OFF:_W3_OFF + 24] = W3p.astype(np.float16)
    wts32 = wts.view(np.float32)
    wts32[0:64, (_B_OFF + 0) // 2] = b1
    wts32[0:48, (_B_OFF + 2) // 2] = b2p
    wts32[0:24, (_B_OFF + 4) // 2] = b3p

    consts = dict(c01=c01, c02=c02, c12=c12, c3s=c3s, ch=ch,
                  b_out=float(b_out[0]), wh=wh, U=U, W3p=W3p, b3p=b3p)
    return x, wts, consts


def _combine(t2_all, consts):
    """t2_all: [48, B] device tower output -> final sigmoid output [B, 1].

    The host finishes the tiny third layer (16k x 48 x 24 sgemm + tanh),
    the cross-scalar recurrence and the sigmoid."""
    t3 = np.tanh(consts["W3p"].T @ t2_all + consts["b3p"][:, None])      # [24, B]
    hd = consts["wh"].astype(np.float64) @ t3.astype(np.float64)         # [B]
    U = consts["U"].astype(np.float64)
    u0, u1, u2, u3 = U[:, 0], U[:, 1], U[:, 2], U[:, 3]
    oneS = ((1.0 + u0) * (1.0 + u1) + consts["c01"]) * (1.0 + u2) \
        + consts["c02"] + consts["c12"]
    lin = oneS * u3 + consts["c3s"] + hd + consts["ch"] + consts["b_out"]
    y = 1.0 / (1.0 + np.exp(-lin))
    return y.reshape(-1, 1).astype(np.float32)


def _run(inputs, trace=False, **spmd_kwargs):
    from concourse.bass_utils import run_bass_kernel_spmd

    x, wts, consts = _prep(inputs)
    nc = _get_nc()

    x16 = x.astype(np.float16).reshape(N_CORES, ROWS, KT, 128)
    in_maps = []
    for c in range(N_CORES):
        # chunk-contiguous packing: for each block and k-chunk, a flat
        # [128, CH, bs] slab so each chunk DMA is one contiguous region
        parts = []
        for off, bs, _, nch in _BLK:
            blk = x16[c, off:off + bs]           # [bs, KT, 128]
            ch = KT // nch
            for j in range(nch):
                parts.append(
                    blk[:, j * ch:(j + 1) * ch, :].transpose(2, 1, 0).ravel())
        in_maps.append({"xt": np.concatenate(parts), "wts": wts})

    res = run_bass_kernel_spmd(
        nc, in_maps, core_ids=list(range(N_CORES)), trace=trace, **spmd_kwargs
    )
    t2_all = np.concatenate([r["out2"] for r in res.results], axis=1)  # [48, B]
    return _combine(t2_all, consts), res


def kernel(**inputs) -> np.ndarray:
    y, _ = _run(inputs, trace=False)
    return y
